# revision 48
# baseline (speedup 1.0000x reference)
"""Trainium2 Bass kernel for JointSelfAttention (B=4,T=2048,C=1024,H=16).

Sharding: 8 cores = 4 batches (data-parallel) x 2 head-groups of 8 heads
(tensor-parallel).  Each core computes qkv for its head group, qk-RMSNorm,
RoPE, causal attention, and a partial c_proj; the host sums the two partial
projections per batch and transposes back.

v4: fp8 DoubleRow on the q/k path.  The cost (and HW stream rate) of a
matmul is its output free size; DoubleRow processes two 128-deep
contraction subtiles per cycle at half the row cost.  q/k-qkv uses genuine
kc pairs; the 64-deep score contraction uses a stride-0 second subtile
(computing 2*q.k, folded into the exp scale).  v/attn@v/proj stay bf16 for
accuracy.  Act runs exp only: squares go to gpsimd+DVE, v-copy to gpsimd,
and the projection DMAs straight from PSUM.
"""

import math
import numpy as np
from contextlib import ExitStack

B, T, C, H, HD = 4, 2048, 1024, 16, 64
HG = 2              # head groups (tensor-parallel dim)
HPG = H // HG       # heads per group = 8
CG = HPG * HD       # channels per group = 512
N_CORES = B * HG
EPS = float(np.finfo(np.float32).eps)
QW = 512            # query window (free dim per attention block)
NQW = T // QW       # 4 windows
NKT = T // 128      # 16 k tiles
NMT = T // 128      # 16 m (token) tiles
NKC = C // 128      # 8 contraction tiles for qkv
WSCALE = 64.0       # host-side premultiplier on w_qk (rmsnorm removes it)

SCORES_FP8 = True   # fp8e4 DoubleRow score matmuls (stride-0 2nd subtile)
QKGEMM_FP8 = True   # fp8e4 DoubleRow for the q/k section of the qkv GEMM
STRIDE0_STAT = True  # stationary kT second subtile via stride-0 AP
SQ_ON_POOL = True   # rmsnorm squares on gpsimd instead of Act
VCOPY_ON_POOL = True  # v psum->sbuf copy on gpsimd instead of Act


def _split_excess_waits(nc, mybir, max_waits=1):
    """This container's walrus only encodes 1 sync-wait per instruction
    ("Too many sync wait commands" in CoreV3 codegen).  Move extra waits to
    preceding NoOps on the same engine."""
    for f in nc.m.functions:
        for bb in f.blocks:
            new_insts = []
            for inst in bb.instructions:
                si = inst.sync_info
                if si is not None and si.on_wait and len(si.on_wait) > max_waits:
                    waits = list(si.on_wait)
                    extra, keep = waits[:-max_waits], waits[-max_waits:]
                    for i in range(0, len(extra), max_waits):
                        nop = mybir.InstNoOp(
                            name=f"{inst.name}-ws{i}", ins=[], outs=[])
                        nop.engine = inst.engine
                        nop.sync_info = mybir.SyncInfo(
                            on_wait=extra[i:i + max_waits], on_update=[])
                        new_insts.append(nop)
                    inst.sync_info = mybir.SyncInfo(
                        on_wait=keep, on_update=list(si.on_update or []))
                new_insts.append(inst)
            bb.instructions.clear()
            bb.instructions.extend(new_insts)


def _build_nc():
    import concourse.bass as bass
    import concourse.tile as tile
    from concourse import mybir
    from concourse.masks import make_identity

    f32 = mybir.dt.float32
    bf16 = mybir.dt.bfloat16
    fp8 = mybir.dt.float8e4
    AF = mybir.ActivationFunctionType
    MUL = mybir.AluOpType.mult
    DR = mybir.MatmulPerfMode.DoubleRow

    nc = bass.Bass("TRN2", debug=False, num_devices=N_CORES)

    qk_dt = fp8 if QKGEMM_FP8 else bf16
    sc_dt = fp8 if SCORES_FP8 else bf16
    # exp scale folds three factors: 1/sqrt(HD); the DoubleRow stride-0
    # doubling (scores arrive as 2*q.k); and the norm factor, computed as
    # (sum q^2)^-0.5 = rsqrt(mean)/8 per side, so scores are 64x small
    ESC = HD / (2.0 * math.sqrt(HD)) if SCORES_FP8 else HD / math.sqrt(HD)

    xt = nc.dram_tensor("xt", [NMT, 128, NKC * 128], bf16, kind="ExternalInput").ap()
    xt8 = None
    if QKGEMM_FP8:
        xt8 = nc.dram_tensor("xt8", [NMT, 128, NKC * 128], fp8, kind="ExternalInput").ap()
    wqk = nc.dram_tensor("wqk", [C, 2 * CG], qk_dt, kind="ExternalInput").ap()
    wv = nc.dram_tensor("wv", [C, CG], bf16, kind="ExternalInput").ap()
    wp = nc.dram_tensor("wp", [CG, C], bf16, kind="ExternalInput").ap()
    cosn = nc.dram_tensor("cosn", [128, NMT, HD // 2], bf16, kind="ExternalInput").ap()
    sinn = nc.dram_tensor("sinn", [128, NMT, HD // 2], bf16, kind="ExternalInput").ap()
    trim = nc.dram_tensor("trim", [128, 128], bf16, kind="ExternalInput").ap()
    out = nc.dram_tensor("o", [C, T], f32, kind="ExternalOutput").ap()

    with tile.TileContext(nc) as tc, ExitStack() as ctx:
        # ---- persistent buffers ----
        persist = ctx.enter_context(tc.tile_pool(name="persist", bufs=1))
        # q/k heads transposed: head h lives at partitions (h%2)*64..+64,
        # free slot h//2 -> [128, 4, T]
        qT = persist.tile([128, HPG // 2, T], sc_dt)
        if STRIDE0_STAT:
            kT = persist.tile([128, HPG // 2, T], sc_dt)
        else:
            # materialized DoubleRow duplicate on the stationary side
            kT = persist.tile([128, 2, HPG // 2, T], sc_dt)
        vaug = persist.tile([128, NKT, HPG, HD + 1], bf16)  # v + ones col
        cos_sb = persist.tile([128, NMT, HD // 2], bf16)
        sin_sb = persist.tile([128, NMT, HD // 2], bf16)
        trim_sb = persist.tile([128, 128], bf16)
        ident = persist.tile([128, 128], bf16)
        eps_sb = persist.tile([128, 1], f32)
        ybuf = persist.tile([128, CG // 128, T], bf16)  # attn out (ch x T)
        wqk_sb = persist.tile([128, NKC, 2 * CG], qk_dt)
        wv_sb = persist.tile([128, NKC, CG], bf16)
        wp_sb = persist.tile([128, CG // 128, C], bf16)
        # tail undivided attn results (+denominator col), per (head, qt)
        y3 = persist.tile([128, HPG, 2, HD + 1], bf16)

        # only what tile 0's q/k sections and rope need rides ahead of the
        # first x DMA; the rest is emitted lazily from p1_gen (the single
        # HWDGE generator serializes every DMA ~0.7us)
        wqk_r = wqk.rearrange("(kc p) n -> p kc n", p=128)
        for kc in range(4):
            nc.scalar.dma_start(wqk_sb[:, kc, :], wqk_r[:, kc, :])
        nc.scalar.dma_start(cos_sb[:], cosn[:])
        nc.scalar.dma_start(sin_sb[:], sinn[:])
        make_identity(nc, ident[:])
        nc.vector.memset(eps_sb[:], EPS)
        nc.gpsimd.memset(
            vaug[:, :, :, HD:HD + 1].rearrange("p a b one -> p (a b one)"), 1.0)

        wv_r = wv.rearrange("(kc p) n -> p kc n", p=128)
        wp_r = wp.rearrange("(kc p) n -> p kc n", p=128)

        def late_weight_dmas(mt):
            # spread behind the early x tiles so those never queue
            if mt == 0:
                for kc in range(4, NKC):
                    nc.scalar.dma_start(wqk_sb[:, kc, :], wqk_r[:, kc, :])
                for kc in range(NKC):
                    nc.scalar.dma_start(wv_sb[:, kc, :], wv_r[:, kc, :])
            elif mt == 1:
                nc.scalar.dma_start(trim_sb[:], trim[:])
            elif mt == 2:
                for kc in range(2):
                    nc.scalar.dma_start(wp_sb[:, kc, :], wp_r[:, kc, :])
            elif mt == 3:
                for kc in range(2, 4):
                    nc.scalar.dma_start(wp_sb[:, kc, :], wp_r[:, kc, :])

        # ---- work generators, woven at sub-unit granularity ----
        # windows: (col0, ncols); w0 split in two so attention starts as soon
        # as the first token tiles land.  Tokens 1536+ run query-tile granular.
        WINS = [(0, 256), (256, 256), (512, 512), (1024, 512), (1536, 256)]
        NTQ = 2              # qt-granular tail query tiles (tokens 1792+)
        NWIN = len(WINS) + 1  # + the qt-granular tail
        state = {"mt_done": -1, "win_done": [0] * NWIN}

        xpool = ctx.enter_context(tc.tile_pool(name="xp", bufs=3))
        pp1 = ctx.enter_context(tc.tile_pool(name="pp1", bufs=2, space="PSUM"))
        tstage = ctx.enter_context(tc.tile_pool(name="ts", bufs=1, space="PSUM"))
        stps = ctx.enter_context(tc.tile_pool(name="stps", bufs=2, space="PSUM"))
        yend = ctx.enter_context(tc.tile_pool(name="yend", bufs=1, space="PSUM"))
        ptpool = ctx.enter_context(tc.tile_pool(name="pt", bufs=3))
        epil = ctx.enter_context(tc.tile_pool(name="epil", bufs=3))
        work = ctx.enter_context(tc.tile_pool(name="work", bufs=3))
        stats = ctx.enter_context(tc.tile_pool(name="stats", bufs=2))
        ostg = ctx.enter_context(tc.tile_pool(name="ostg", bufs=3))

        def kdr(h, kt):
            """Stationary score operand [64, 2, 128] for DoubleRow."""
            po, tr = (h % 2) * 64, h // 2
            if STRIDE0_STAT:
                base = kT[po:po + 64, tr, kt * 128:(kt + 1) * 128]
                return bass.AP(tensor=base.tensor, offset=base.offset,
                               ap=[base.ap[0], [0, 2], base.ap[-1]])
            return kT[po:po + 64, :, tr, kt * 128:(kt + 1) * 128]

        def qdr(h, c0, c1):
            """Moving score operand [64, 2, c1-c0] (stride-0 2nd subtile)."""
            po, tr = (h % 2) * 64, h // 2
            base = qT[po:po + 64, tr, c0:c1]
            return bass.AP(tensor=base.tensor, offset=base.offset,
                           ap=[base.ap[0], [0, 2], base.ap[-1]])

        def score_mm(st_out, h, kt, c0, c1):
            po, tr = (h % 2) * 64, h // 2
            if SCORES_FP8:
                nc.tensor.matmul(st_out, kdr(h, kt), qdr(h, c0, c1),
                                 start=True, stop=True, perf_mode=DR)
            else:
                nc.tensor.matmul(
                    st_out,
                    kT[po:po + 64, tr, kt * 128:(kt + 1) * 128],
                    qT[po:po + 64, tr, c0:c1], start=True, stop=True)

        def transpose_tile_pieces(pmt, pqkn):
            # delayed by 1-2 tiles so the PE never waits on the rope chain.
            # All 8 transposes first, then the copies: a copy of section 0
            # must not gate section 1's transposes (same tile -> the tile
            # framework would serialize them).
            tp = tstage.tile([128, 2, HPG // 2, 128], bf16, tag="tp")
            for sec in range(2):
                for j in range(HPG // 2):
                    src = pqkn[:, sec * CG + 2 * j * HD:
                               sec * CG + (2 * j + 2) * HD]
                    nc.tensor.transpose(
                        tp[:, sec, j, :],
                        src.rearrange("p (two d) -> p two d", two=2),
                        ident[:])
            yield
            nc.vector.tensor_copy(
                qT[:, :, pmt * 128:(pmt + 1) * 128], tp[:, 0])
            if STRIDE0_STAT:
                nc.vector.tensor_copy(
                    kT[:, :, pmt * 128:(pmt + 1) * 128], tp[:, 1])
            else:
                for j2 in range(2):
                    nc.vector.tensor_copy(
                        kT[:, j2, :, pmt * 128:(pmt + 1) * 128], tp[:, 1])
            state["mt_done"] = pmt
            yield

        def p1_gen():
            pending = []  # [(mt, qkn)] awaiting transposes
            for mt in range(NMT):
                xt_sb = xpool.tile([128, NKC, 128], bf16, tag="xt")
                if QKGEMM_FP8:
                    # fp8 first: the q/k matmuls need it before v needs bf16
                    xt8_sb = xpool.tile([128, NKC, 128], fp8, tag="xt8")
                    nc.sync.dma_start(
                        xt8_sb[:].rearrange("p kc t -> p (kc t)"), xt8[mt])
                    qk_x = xt8_sb
                else:
                    qk_x = xt_sb
                nc.sync.dma_start(
                    xt_sb[:].rearrange("p kc t -> p (kc t)"), xt[mt])
                late_weight_dmas(mt)

                qkn = work.tile([128, 2 * CG], bf16, tag="qkn")

                def stats_norm_rope(sec, s_ps):
                    # copy-first: one DVE read frees the PSUM slot; all the
                    # stats run on the cheap (4x-mode) bf16 SBUF copy
                    qc = work.tile([128, CG], bf16, tag="qc")
                    nc.vector.tensor_copy(qc[:], s_ps[:])
                    sq = work.tile([128, CG], bf16, tag="sq")
                    nc.vector.tensor_tensor(sq[:], qc[:], qc[:], op=MUL)
                    ss = stats.tile([128, HPG], bf16, tag="ss")
                    with nc.allow_low_precision(
                            reason="bf16 sumsq feeds an rmsnorm factor"):
                        nc.vector.tensor_reduce(
                            ss[:], sq[:].rearrange("p (h d) -> p h d", d=HD),
                            axis=mybir.AxisListType.X, op=mybir.AluOpType.add)
                    # rr = (sum q^2 + HD*eps)^-0.5 = rsqrt(mean q^2)/8; the
                    # /8 per side folds into the exp scale.  On gpsimd so the
                    # Act engine stays exp-only (a waiting Sqrt would block
                    # Act's in-order queue).
                    rrb = stats.tile([128, HPG], bf16, tag="rrb")
                    nc.gpsimd.tensor_scalar(
                        rrb[:], ss[:], HD * EPS, -0.5,
                        op0=mybir.AluOpType.add, op1=mybir.AluOpType.pow)
                    rr_b = bass.AP(
                        tensor=rrb.tensor, offset=rrb.offset,
                        ap=[rrb.ap[0], [1, HPG], [0, HD]])
                    qs = qkn[:, sec * CG:(sec + 1) * CG]
                    nc.vector.tensor_tensor(
                        qs.rearrange("p (h d) -> p h d", d=HD),
                        qc[:].rearrange("p (h d) -> p h d", d=HD),
                        rr_b, op=MUL)
                    # rope in place: y1 = x1 c + x2 s ; y2 = x2 c - x1 s
                    base = qs.rearrange("p (h two d) -> p h two d",
                                        two=2, d=HD // 2)
                    x1, x2 = base[:, :, 0, :], base[:, :, 1, :]
                    # cos/sin broadcast over heads AND both rope halves
                    cb2 = bass.AP(
                        tensor=cos_sb.tensor,
                        offset=cos_sb.offset + mt * (HD // 2),
                        ap=[cos_sb.ap[0], [0, HPG], [0, 2], [1, HD // 2]])
                    sb2 = bass.AP(
                        tensor=sin_sb.tensor,
                        offset=sin_sb.offset + mt * (HD // 2),
                        ap=[sin_sb.ap[0], [0, HPG], [0, 2], [1, HD // 2]])
                    tc = work.tile([128, HPG, 2, HD // 2], bf16, tag="rtc")
                    ts = work.tile([128, HPG, 2, HD // 2], bf16, tag="rts")
                    nc.vector.tensor_tensor(tc[:], base, cb2, op=MUL)
                    nc.vector.tensor_tensor(ts[:], base, sb2, op=MUL)
                    nc.vector.tensor_add(x1, tc[:, :, 0, :], ts[:, :, 1, :])
                    nc.vector.tensor_sub(x2, tc[:, :, 1, :], ts[:, :, 0, :])

                # q, k, v rotate through the pp1 PSUM slots; stats/norm/rope
                # drain while later chunks and woven attention pieces run
                for sec in range(2):
                    s_ps = pp1.tile([128, CG], f32, tag="ps1")
                    if QKGEMM_FP8:
                        for g in range(2):
                            nc.tensor.matmul(
                                s_ps[:], qk_x[:, 2 * g:2 * g + 2, :],
                                wqk_sb[:, 2 * g:2 * g + 2,
                                       sec * CG:(sec + 1) * CG],
                                start=(g == 0), stop=False, perf_mode=DR)
                        yield
                        for g in range(2, 4):
                            nc.tensor.matmul(
                                s_ps[:], qk_x[:, 2 * g:2 * g + 2, :],
                                wqk_sb[:, 2 * g:2 * g + 2,
                                       sec * CG:(sec + 1) * CG],
                                start=False, stop=(g == 3), perf_mode=DR)
                    else:
                        for kc in range(4):
                            nc.tensor.matmul(
                                s_ps[:], qk_x[:, kc, :],
                                wqk_sb[:, kc, sec * CG:(sec + 1) * CG],
                                start=(kc == 0), stop=False)
                        yield
                        for kc in range(4, NKC):
                            nc.tensor.matmul(
                                s_ps[:], qk_x[:, kc, :],
                                wqk_sb[:, kc, sec * CG:(sec + 1) * CG],
                                start=False, stop=(kc == NKC - 1))
                    stats_norm_rope(sec, s_ps)
                    yield

                v_ps = pp1.tile([128, CG], f32, tag="ps1")
                for kc in range(4):
                    nc.tensor.matmul(
                        v_ps[:], xt_sb[:, kc, :], wv_sb[:, kc, :],
                        start=(kc == 0), stop=False)
                yield
                for kc in range(4, NKC):
                    nc.tensor.matmul(
                        v_ps[:], xt_sb[:, kc, :], wv_sb[:, kc, :],
                        start=False, stop=(kc == NKC - 1))
                # v -> vaug (strided per-head copy, leaves ones col intact)
                veng = nc.gpsimd if VCOPY_ON_POOL else nc.scalar
                veng.tensor_copy(
                    vaug[:, mt, :, 0:HD],
                    v_ps[:].rearrange("p (h d) -> p h d", d=HD))
                pending.append((mt, qkn))
                yield
                # transposes lag 1-2 tiles, emitted AFTER this tile's
                # sections: the PE must never head-of-line block on the
                # rope chain with ready qkv matmuls queued behind it
                lag = 1 if mt <= 3 else 2
                while pending and mt - pending[0][0] >= lag:
                    yield from transpose_tile_pieces(*pending.pop(0))
            while pending:
                yield from transpose_tile_pieces(*pending.pop(0))

        def av_win_pieces(h, wi, c0, sz, pt):
            """attn@v + epilogue for a windowed unit, as yieldable pieces."""
            po, tr = (h % 2) * 64, h // 2
            nq = sz // 128
            qt0 = c0 // 128
            yT = yend.tile([128, nq, HD + 1], f32, tag="ye")
            for i in range(nq):
                qt = qt0 + i
                for kt in range(qt + 1):
                    nc.tensor.matmul(
                        yT[:, i, :],
                        pt[:, kt, i * 128:(i + 1) * 128],
                        vaug[:, kt, h, :],
                        start=(kt == 0), stop=(kt == qt))
                if i % 2 == 1 and i != nq - 1:
                    yield
            # divide by denominator (col 64, per-partition, broadcast over d)
            den_b = bass.AP(tensor=yT.tensor, offset=yT.offset + HD,
                            ap=[yT.ap[0], [HD + 1, nq], [0, HD]])
            ysb = epil.tile([128, nq, HD], bf16, tag=f"ysb{nq}")
            nc.gpsimd.tensor_tensor(ysb[:], yT[:, :, 0:HD], den_b,
                                    op=mybir.AluOpType.divide)
            yield
            # back to (ch x T) for the projection
            ytp = yend.tile([64, nq, 128], bf16, tag="ye")
            for i in range(nq):
                nc.tensor.transpose(ytp[:, i, :], ysb[:, i, :], ident[:])
            nc.vector.tensor_copy(
                ybuf[po:po + 64, tr, c0:c0 + sz]
                .rearrange("p (i t) -> p i t", t=128),
                ytp[:])
            state["win_done"][wi] += 1
            yield

        def av_qt_pieces(h, qt, pt3, last):
            """attn@v for a qt-granular unit (tail); epilogue inline after
            the last qt so each head's tail completes as early as possible."""
            yT3 = yend.tile([128, HD + 1], f32, tag="ye")
            for kt in range(qt + 1):
                nc.tensor.matmul(
                    yT3[:], pt3[:, kt, :], vaug[:, kt, h, :],
                    start=(kt == 0), stop=(kt == qt))
            # park the undivided result (and its denominator) in SBUF.
            # Tail pieces run on DVE: the Pool queue backlog would otherwise
            # hold the final barrier
            eng = nc.vector if qt == NMT - 1 else nc.gpsimd
            eng.tensor_copy(y3[:, h, qt - (NMT - NTQ), :], yT3[:])
            yield
            if not last:
                return
            po, tr = (h % 2) * 64, h // 2
            y3h = y3[:, h]
            den_b = bass.AP(tensor=y3h.tensor, offset=y3h.offset + HD,
                            ap=[y3h.ap[0], [HD + 1, NTQ], [0, HD]])
            ysb = epil.tile([128, NTQ, HD], bf16, tag="ysbt")
            nc.vector.tensor_tensor(ysb[:], y3[:, h, :, 0:HD], den_b,
                                    op=mybir.AluOpType.divide)
            yield
            ytp = yend.tile([64, NTQ, 128], bf16, tag="ye")
            for i in range(NTQ):
                nc.tensor.transpose(ytp[:, i, :], ysb[:, i, :], ident[:])
            nc.vector.tensor_copy(
                ybuf[po:po + 64, tr, T - NTQ * 128:T]
                .rearrange("p (i t) -> p i t", t=128),
                ytp[:])
            state["win_done"][NWIN - 1] += 1
            yield

        def attn_gen():
            # software pipeline: while unit k's scores+exp stream, unit k-1's
            # attn@v chains run between them, so the PE never parks waiting
            # for the Act engine's exp of the current unit.
            pend = None  # av piece generator of the previous unit

            def drain_one():
                nonlocal pend
                if pend is not None:
                    try:
                        next(pend)
                    except StopIteration:
                        pend = None

            for wi, (c0, sz) in enumerate(WINS):
                n_kt = (c0 + sz) // 128
                qt0 = c0 // 128
                # one tile of lead (copies drain before the gated scores),
                # except early windows where attention must start ASAP
                need = min(n_kt, NMT - 1) if n_kt > 4 else n_kt - 1
                for h in range(HPG):
                    while state["mt_done"] < need:
                        if pend is not None:
                            drain_one()
                        yield "wait"
                    pt = ptpool.tile([128, 14, QW], bf16, tag="pt")
                    for g in range((n_kt + 1) // 2):
                        st = stps.tile([128, 2, QW], f32, tag="st")
                        kts = [kt for kt in (2 * g, 2 * g + 1) if kt < n_kt]
                        for j, kt in enumerate(kts):
                            d = kt - qt0
                            col0 = d * 128 if d >= 0 else 0
                            score_mm(st[:, j, col0:col0 + (sz - col0)],
                                     h, kt, c0 + col0, c0 + sz)
                        # batched exp; stale PSUM cols left of the causal
                        # edge are exp'd but never read
                        ec = min(max(0, (2 * g - qt0) * 128), sz - 128)
                        nc.scalar.activation(
                            pt[:, 2 * g:2 * g + len(kts), ec:sz],
                            st[:, 0:len(kts), ec:sz],
                            AF.Exp, scale=ESC)
                        for kt in kts:
                            d = kt - qt0
                            if d >= 0:
                                col0 = d * 128
                                nc.gpsimd.tensor_tensor(
                                    pt[:, kt, col0:col0 + 128],
                                    pt[:, kt, col0:col0 + 128], trim_sb[:],
                                    op=MUL)
                        drain_one()
                        yield
                    while pend is not None:
                        drain_one()
                        yield
                    pend = av_win_pieces(h, wi, c0, sz, pt)

            # ---- tail: query-tile granular ----
            for qt in range(NMT - NTQ, NMT):
                for h in range(HPG):
                    while state["mt_done"] < qt:
                        if pend is not None:
                            drain_one()
                        yield "wait"
                    pt3 = ptpool.tile([128, NKT, 128], bf16, tag="pt3")
                    for b in range(2):
                        kts = list(range(8 * b, min(8 * b + 8, qt + 1)))
                        st = stps.tile([128, 8, 128], f32, tag="st")
                        for i, kt in enumerate(kts):
                            score_mm(st[:, i, :], h, kt,
                                     qt * 128, (qt + 1) * 128)
                        nc.scalar.activation(
                            pt3[:, 8 * b:8 * b + len(kts), :],
                            st[:, 0:len(kts), :],
                            AF.Exp, scale=ESC)
                        if qt in kts:
                            nc.gpsimd.tensor_tensor(
                                pt3[:, qt, :], pt3[:, qt, :], trim_sb[:],
                                op=MUL)
                        drain_one()
                        yield
                    while pend is not None:
                        drain_one()
                        yield
                    pend = av_qt_pieces(h, qt, pt3, last=(qt == NMT - 1))
            while pend is not None:
                drain_one()
                yield

        def proj_gen():
            pw = WINS + [(T - NTQ * 128, NTQ * 128)]
            for (wi, (c0, sz)), mo in [((wi, w), mo) for wi, w in enumerate(pw)
                                       for mo in range(C // 128)]:
                while state["win_done"][wi] < HPG:
                    yield "wait"
                po_ps = pp1.tile([128, QW], f32, tag="ps1")
                for kc in range(CG // 128):
                    nc.tensor.matmul(
                        po_ps[:, 0:sz],
                        wp_sb[:, kc, mo * 128:(mo + 1) * 128],
                        ybuf[:, kc, c0:c0 + sz],
                        start=(kc == 0), stop=(kc == CG // 128 - 1))
                ot = ostg.tile([128, QW], f32)
                # alternate engines so neither queue serializes the drain;
                # the last window avoids Pool (barrier waits on its backlog)
                ceng = nc.gpsimd if (mo % 2 == 0 and wi < NWIN - 1) \
                    else nc.vector
                ceng.tensor_copy(ot[:, 0:sz], po_ps[:, 0:sz])
                nc.sync.dma_start(
                    out[mo * 128:(mo + 1) * 128, c0:c0 + sz],
                    ot[:, 0:sz])
                yield

        # weave: one p1/proj piece, then up to two attention pieces
        g_p1, g_at, g_pj = p1_gen(), attn_gen(), proj_gen()

        def step(g):
            if g is None:
                return None, False
            try:
                r = next(g)
                return g, r != "wait"
            except StopIteration:
                return None, False

        while g_p1 is not None or g_at is not None or g_pj is not None:
            progressed = False
            if g_p1 is not None:
                g_p1, ok = step(g_p1)
                progressed |= ok
            else:
                g_pj, ok = step(g_pj)
                progressed |= ok
            for _ in range(2):
                g_at, ok = step(g_at)
                progressed |= ok
                if g_at is None:
                    break
            if not progressed and g_p1 is None and g_at is None:
                # drain remaining proj
                while g_pj is not None:
                    g_pj, _ = step(g_pj)

    _split_excess_waits(nc, mybir)
    return nc


_NC_CACHE = {}


def _get_nc():
    if "nc" not in _NC_CACHE:
        _NC_CACHE["nc"] = _build_nc()
    return _NC_CACHE["nc"]


def _host_inputs(x, w_attn, w_proj):
    import ml_dtypes
    bf = ml_dtypes.bfloat16
    f8 = ml_dtypes.float8_e4m3
    inv_freq = 1.0 / (10000.0 ** (np.arange(0, HD, 2, dtype=np.float32) / HD))
    t = np.arange(T, dtype=np.float32)
    freqs = np.outer(t, inv_freq)
    cos = np.cos(freqs).astype(bf)
    sin = np.sin(freqs).astype(bf)
    cosn = np.ascontiguousarray(cos.reshape(NMT, 128, HD // 2).transpose(1, 0, 2))
    sinn = np.ascontiguousarray(sin.reshape(NMT, 128, HD // 2).transpose(1, 0, 2))
    trim = np.triu(np.ones((128, 128), dtype=np.float32)).astype(bf)

    in_maps = []
    for b in range(B):
        xT = np.ascontiguousarray(x[b].T)  # (C, T)
        # [mt, ch-in-chunk, kc*128+tok]: one contiguous DMA per token tile,
        # partitions carry the contraction channels
        xt_f = np.ascontiguousarray(
            xT.reshape(NKC, 128, NMT, 128).transpose(2, 1, 0, 3)
        ).reshape(NMT, 128, NKC * 128)
        xt = xt_f.astype(bf)
        xt8 = xt_f.astype(f8)
        for hg in range(HG):
            qr = slice(hg * CG, (hg + 1) * CG)
            kr = slice(C + hg * CG, C + (hg + 1) * CG)
            vr = slice(2 * C + hg * CG, 2 * C + (hg + 1) * CG)
            wqk_f = np.ascontiguousarray(
                np.concatenate([w_attn[qr], w_attn[kr]], axis=0).T)
            if QKGEMM_FP8:
                wqk_h = (wqk_f * WSCALE).astype(f8)
            else:
                wqk_h = wqk_f.astype(bf)
            wv = np.ascontiguousarray(w_attn[vr].T).astype(bf)
            wp = np.ascontiguousarray(w_proj[:, hg * CG:(hg + 1) * CG].T).astype(bf)
            m = {
                "xt": xt, "wqk": wqk_h, "wv": wv, "wp": wp,
                "cosn": cosn, "sinn": sinn, "trim": trim,
            }
            if QKGEMM_FP8:
                m["xt8"] = xt8
            in_maps.append(m)
    return in_maps


def kernel(x, w_attn, w_proj, _profile=False):
    from concourse.bass_utils import run_bass_kernel_spmd
    nc = _get_nc()
    in_maps = _host_inputs(
        np.asarray(x, dtype=np.float32),
        np.asarray(w_attn, dtype=np.float32),
        np.asarray(w_proj, dtype=np.float32))
    res = run_bass_kernel_spmd(nc, in_maps, core_ids=list(range(N_CORES)),
                               trace=_profile)
    out = np.empty((B, T, C), dtype=np.float32)
    for b in range(B):
        acc = res.results[2 * b]["o"] + res.results[2 * b + 1]["o"]
        out[b] = acc.T
    if _profile:
        return out, res
    return out


# revision 57
# speedup vs baseline: 1.0584x; 1.0584x over previous
"""Trainium2 Bass kernel for JointSelfAttention (B=4,T=2048,C=1024,H=16).

Sharding: 8 cores = 4 batches (data-parallel) x 2 head-groups of 8 heads
(tensor-parallel).  Each core computes qkv for its head group, qk-RMSNorm,
RoPE, causal attention, and a partial c_proj; the host sums the two partial
projections per batch and transposes back.

v4: fp8 DoubleRow on the q/k path.  The cost (and HW stream rate) of a
matmul is its output free size; DoubleRow processes two 128-deep
contraction subtiles per cycle at half the row cost.  q/k-qkv uses genuine
kc pairs; the 64-deep score contraction uses a stride-0 second subtile
(computing 2*q.k, folded into the exp scale).  v/attn@v/proj stay bf16 for
accuracy.  Act runs exp only: squares go to gpsimd+DVE, v-copy to gpsimd,
and the projection DMAs straight from PSUM.
"""

import math
import os
import numpy as np
from contextlib import ExitStack

B, T, C, H, HD = 4, 2048, 1024, 16, 64
HG = 2              # head groups (tensor-parallel dim)
HPG = H // HG       # heads per group = 8
CG = HPG * HD       # channels per group = 512
N_CORES = B * HG
EPS = float(np.finfo(np.float32).eps)
QW = 512            # query window (free dim per attention block)
NQW = T // QW       # 4 windows
NKT = T // 128      # 16 k tiles
NMT = T // 128      # 16 m (token) tiles
NKC = C // 128      # 8 contraction tiles for qkv
WSCALE = 64.0       # host-side premultiplier on w_qk (rmsnorm removes it)

# schedule-tuning knobs (env-overridable for sweeps)
TRANS_AFTER = int(os.environ.get("K_TRANS_AFTER", "0"))
LEAD_NKT = int(os.environ.get("K_LEAD_NKT", "8"))  # lead when n_kt > this
LAG = int(os.environ.get("K_LAG", "1"))
ATTN_PER_ROUND = int(os.environ.get("K_APR", "2"))

SCORES_FP8 = True   # fp8e4 DoubleRow score matmuls (stride-0 2nd subtile)
QKGEMM_FP8 = True   # fp8e4 DoubleRow for the q/k section of the qkv GEMM
STRIDE0_STAT = True  # stationary kT second subtile via stride-0 AP
SQ_ON_POOL = True   # rmsnorm squares on gpsimd instead of Act
VCOPY_ON_POOL = True  # v psum->sbuf copy on gpsimd instead of Act


def _split_excess_waits(nc, mybir, max_waits=1):
    """This container's walrus only encodes 1 sync-wait per instruction
    ("Too many sync wait commands" in CoreV3 codegen).  Move extra waits to
    preceding NoOps on the same engine."""
    for f in nc.m.functions:
        for bb in f.blocks:
            new_insts = []
            for inst in bb.instructions:
                si = inst.sync_info
                if si is not None and si.on_wait and len(si.on_wait) > max_waits:
                    waits = list(si.on_wait)
                    extra, keep = waits[:-max_waits], waits[-max_waits:]
                    for i in range(0, len(extra), max_waits):
                        nop = mybir.InstNoOp(
                            name=f"{inst.name}-ws{i}", ins=[], outs=[])
                        nop.engine = inst.engine
                        nop.sync_info = mybir.SyncInfo(
                            on_wait=extra[i:i + max_waits], on_update=[])
                        new_insts.append(nop)
                    inst.sync_info = mybir.SyncInfo(
                        on_wait=keep, on_update=list(si.on_update or []))
                new_insts.append(inst)
            bb.instructions.clear()
            bb.instructions.extend(new_insts)


def _build_nc():
    import concourse.bass as bass
    import concourse.tile as tile
    from concourse import mybir
    from concourse.masks import make_identity

    f32 = mybir.dt.float32
    bf16 = mybir.dt.bfloat16
    fp8 = mybir.dt.float8e4
    AF = mybir.ActivationFunctionType
    MUL = mybir.AluOpType.mult
    DR = mybir.MatmulPerfMode.DoubleRow

    nc = bass.Bass("TRN2", debug=False, num_devices=N_CORES)

    qk_dt = fp8 if QKGEMM_FP8 else bf16
    sc_dt = fp8 if SCORES_FP8 else bf16
    # exp scale folds three factors: 1/sqrt(HD); the DoubleRow stride-0
    # doubling (scores arrive as 2*q.k); and the norm factor, computed as
    # (sum q^2)^-0.5 = rsqrt(mean)/8 per side, so scores are 64x small
    ESC = HD / (2.0 * math.sqrt(HD)) if SCORES_FP8 else HD / math.sqrt(HD)

    xt = nc.dram_tensor("xt", [NMT, 128, NKC * 128], bf16, kind="ExternalInput").ap()
    xt8 = None
    if QKGEMM_FP8:
        xt8 = nc.dram_tensor("xt8", [NMT, 128, NKC * 128], fp8, kind="ExternalInput").ap()
    wqk = nc.dram_tensor("wqk", [C, 2 * CG], qk_dt, kind="ExternalInput").ap()
    wv = nc.dram_tensor("wv", [C, CG], bf16, kind="ExternalInput").ap()
    wp = nc.dram_tensor("wp", [CG, C], bf16, kind="ExternalInput").ap()
    cosn = nc.dram_tensor("cosn", [128, NMT, HD // 2], bf16, kind="ExternalInput").ap()
    sinn = nc.dram_tensor("sinn", [128, NMT, HD // 2], bf16, kind="ExternalInput").ap()
    trim = nc.dram_tensor("trim", [128, 128], bf16, kind="ExternalInput").ap()
    out = nc.dram_tensor("o", [C, T], f32, kind="ExternalOutput").ap()

    with tile.TileContext(nc) as tc, ExitStack() as ctx:
        # ---- persistent buffers ----
        persist = ctx.enter_context(tc.tile_pool(name="persist", bufs=1))
        # q/k heads transposed: head h lives at partitions (h%2)*64..+64,
        # free slot h//2 -> [128, 4, T]
        qT = persist.tile([128, HPG // 2, T], sc_dt)
        if STRIDE0_STAT:
            kT = persist.tile([128, HPG // 2, T], sc_dt)
        else:
            # materialized DoubleRow duplicate on the stationary side
            kT = persist.tile([128, 2, HPG // 2, T], sc_dt)
        vaug = persist.tile([128, NKT, HPG, HD + 1], bf16)  # v + ones col
        cos_sb = persist.tile([128, NMT, HD // 2], bf16)
        sin_sb = persist.tile([128, NMT, HD // 2], bf16)
        trim_sb = persist.tile([128, 128], bf16)
        ident = persist.tile([128, 128], bf16)
        eps_sb = persist.tile([128, 1], f32)
        ybuf = persist.tile([128, CG // 128, T], bf16)  # attn out (ch x T)
        wqk_sb = persist.tile([128, NKC, 2 * CG], qk_dt)
        wv_sb = persist.tile([128, NKC, CG], bf16)
        wp_sb = persist.tile([128, CG // 128, C], bf16)
        # tail undivided attn results (+denominator col), per (head, qt)
        y3 = persist.tile([128, HPG, 2, HD + 1], bf16)

        # only what tile 0's q/k sections and rope need rides ahead of the
        # first x DMA; the rest is emitted lazily from p1_gen (the single
        # HWDGE generator serializes every DMA ~0.7us)
        wqk_r = wqk.rearrange("(kc p) n -> p kc n", p=128)
        for kc in range(4):
            nc.scalar.dma_start(wqk_sb[:, kc, :], wqk_r[:, kc, :])
        nc.scalar.dma_start(cos_sb[:], cosn[:])
        nc.scalar.dma_start(sin_sb[:], sinn[:])
        make_identity(nc, ident[:])
        nc.vector.memset(eps_sb[:], EPS)
        nc.gpsimd.memset(
            vaug[:, :, :, HD:HD + 1].rearrange("p a b one -> p (a b one)"), 1.0)

        wv_r = wv.rearrange("(kc p) n -> p kc n", p=128)
        wp_r = wp.rearrange("(kc p) n -> p kc n", p=128)

        def late_weight_dmas(mt):
            # spread behind the early x tiles so those never queue
            if mt == 0:
                for kc in range(4, NKC):
                    nc.scalar.dma_start(wqk_sb[:, kc, :], wqk_r[:, kc, :])
                for kc in range(NKC):
                    nc.scalar.dma_start(wv_sb[:, kc, :], wv_r[:, kc, :])
            elif mt == 1:
                nc.scalar.dma_start(trim_sb[:], trim[:])
            elif mt == 2:
                for kc in range(2):
                    nc.scalar.dma_start(wp_sb[:, kc, :], wp_r[:, kc, :])
            elif mt == 3:
                for kc in range(2, 4):
                    nc.scalar.dma_start(wp_sb[:, kc, :], wp_r[:, kc, :])

        # ---- work generators, woven at sub-unit granularity ----
        # windows: (col0, ncols); w0 split in two so attention starts as soon
        # as the first token tiles land.  Tokens 1536+ run query-tile granular.
        WINS = [(0, 256), (256, 256), (512, 512), (1024, 512), (1536, 256)]
        NTQ = 2              # qt-granular tail query tiles (tokens 1792+)
        NWIN = len(WINS) + 1  # + the qt-granular tail
        state = {"mt_done": -1, "win_done": [0] * NWIN}

        xpool = ctx.enter_context(tc.tile_pool(name="xp", bufs=3))
        pp1 = ctx.enter_context(tc.tile_pool(name="pp1", bufs=2, space="PSUM"))
        tstage = ctx.enter_context(tc.tile_pool(name="ts", bufs=1, space="PSUM"))
        stps = ctx.enter_context(tc.tile_pool(name="stps", bufs=2, space="PSUM"))
        yend = ctx.enter_context(tc.tile_pool(name="yend", bufs=1, space="PSUM"))
        ptpool = ctx.enter_context(tc.tile_pool(name="pt", bufs=3))
        epil = ctx.enter_context(tc.tile_pool(name="epil", bufs=3))
        work = ctx.enter_context(tc.tile_pool(name="work", bufs=3))
        stats = ctx.enter_context(tc.tile_pool(name="stats", bufs=2))
        ostg = ctx.enter_context(tc.tile_pool(name="ostg", bufs=3))

        def kdr(h, kt):
            """Stationary score operand [64, 2, 128] for DoubleRow."""
            po, tr = (h % 2) * 64, h // 2
            if STRIDE0_STAT:
                base = kT[po:po + 64, tr, kt * 128:(kt + 1) * 128]
                return bass.AP(tensor=base.tensor, offset=base.offset,
                               ap=[base.ap[0], [0, 2], base.ap[-1]])
            return kT[po:po + 64, :, tr, kt * 128:(kt + 1) * 128]

        def qdr(h, c0, c1):
            """Moving score operand [64, 2, c1-c0] (stride-0 2nd subtile)."""
            po, tr = (h % 2) * 64, h // 2
            base = qT[po:po + 64, tr, c0:c1]
            return bass.AP(tensor=base.tensor, offset=base.offset,
                           ap=[base.ap[0], [0, 2], base.ap[-1]])

        def score_mm(st_out, h, kt, c0, c1):
            po, tr = (h % 2) * 64, h // 2
            if SCORES_FP8:
                nc.tensor.matmul(st_out, kdr(h, kt), qdr(h, c0, c1),
                                 start=True, stop=True, perf_mode=DR)
            else:
                nc.tensor.matmul(
                    st_out,
                    kT[po:po + 64, tr, kt * 128:(kt + 1) * 128],
                    qT[po:po + 64, tr, c0:c1], start=True, stop=True)

        def transpose_tile_pieces(pmt, pqkn):
            # delayed by 1-2 tiles so the PE never waits on the rope chain.
            # All 8 transposes first, then the copies: a copy of section 0
            # must not gate section 1's transposes (same tile -> the tile
            # framework would serialize them).
            tp = tstage.tile([128, 2, HPG // 2, 128], bf16, tag="tp")
            for sec in range(2):
                for j in range(HPG // 2):
                    src = pqkn[:, sec * CG + 2 * j * HD:
                               sec * CG + (2 * j + 2) * HD]
                    nc.tensor.transpose(
                        tp[:, sec, j, :],
                        src.rearrange("p (two d) -> p two d", two=2),
                        ident[:])
            yield
            # on Pool: the DVE is the scarce engine during qkv production
            nc.gpsimd.tensor_copy(
                qT[:, :, pmt * 128:(pmt + 1) * 128], tp[:, 0])
            if STRIDE0_STAT:
                nc.gpsimd.tensor_copy(
                    kT[:, :, pmt * 128:(pmt + 1) * 128], tp[:, 1])
            else:
                for j2 in range(2):
                    nc.gpsimd.tensor_copy(
                        kT[:, j2, :, pmt * 128:(pmt + 1) * 128], tp[:, 1])
            state["mt_done"] = pmt
            yield

        def p1_gen():
            pending = []  # [(mt, qkn)] awaiting transposes
            for mt in range(NMT):
                xt_sb = xpool.tile([128, NKC, 128], bf16, tag="xt")
                if QKGEMM_FP8:
                    # fp8 first: the q/k matmuls need it before v needs bf16
                    xt8_sb = xpool.tile([128, NKC, 128], fp8, tag="xt8")
                    nc.sync.dma_start(
                        xt8_sb[:].rearrange("p kc t -> p (kc t)"), xt8[mt])
                    qk_x = xt8_sb
                else:
                    qk_x = xt_sb
                nc.sync.dma_start(
                    xt_sb[:].rearrange("p kc t -> p (kc t)"), xt[mt])
                late_weight_dmas(mt)

                qkn = work.tile([128, 2 * CG], bf16, tag="qkn")

                def stats_norm_rope(sec, s_ps):
                    # copy-first: one DVE read frees the PSUM slot; all the
                    # stats run on the cheap (4x-mode) bf16 SBUF copy
                    qc = work.tile([128, CG], bf16, tag="qc")
                    nc.vector.tensor_copy(qc[:], s_ps[:])
                    sq = work.tile([128, CG], bf16, tag="sq")
                    nc.vector.tensor_tensor(sq[:], qc[:], qc[:], op=MUL)
                    ss = stats.tile([128, HPG], bf16, tag="ss")
                    with nc.allow_low_precision(
                            reason="bf16 sumsq feeds an rmsnorm factor"):
                        nc.vector.tensor_reduce(
                            ss[:], sq[:].rearrange("p (h d) -> p h d", d=HD),
                            axis=mybir.AxisListType.X, op=mybir.AluOpType.add)
                    # rr = (sum q^2 + HD*eps)^-0.5 = rsqrt(mean q^2)/8; the
                    # /8 per side folds into the exp scale.  On gpsimd so the
                    # Act engine stays exp-only (a waiting Sqrt would block
                    # Act's in-order queue).
                    rrb = stats.tile([128, HPG], bf16, tag="rrb")
                    nc.gpsimd.tensor_scalar(
                        rrb[:], ss[:], HD * EPS, -0.5,
                        op0=mybir.AluOpType.add, op1=mybir.AluOpType.pow)
                    rr_b = bass.AP(
                        tensor=rrb.tensor, offset=rrb.offset,
                        ap=[rrb.ap[0], [1, HPG], [0, HD]])
                    qs = qkn[:, sec * CG:(sec + 1) * CG]
                    nc.vector.tensor_tensor(
                        qs.rearrange("p (h d) -> p h d", d=HD),
                        qc[:].rearrange("p (h d) -> p h d", d=HD),
                        rr_b, op=MUL)
                    # rope in place: y1 = x1 c + x2 s ; y2 = x2 c - x1 s
                    base = qs.rearrange("p (h two d) -> p h two d",
                                        two=2, d=HD // 2)
                    x1, x2 = base[:, :, 0, :], base[:, :, 1, :]
                    # cos/sin broadcast over heads AND both rope halves
                    cb2 = bass.AP(
                        tensor=cos_sb.tensor,
                        offset=cos_sb.offset + mt * (HD // 2),
                        ap=[cos_sb.ap[0], [0, HPG], [0, 2], [1, HD // 2]])
                    sb2 = bass.AP(
                        tensor=sin_sb.tensor,
                        offset=sin_sb.offset + mt * (HD // 2),
                        ap=[sin_sb.ap[0], [0, HPG], [0, 2], [1, HD // 2]])
                    tc = work.tile([128, HPG, 2, HD // 2], bf16, tag="rtc")
                    ts = work.tile([128, HPG, 2, HD // 2], bf16, tag="rts")
                    nc.vector.tensor_tensor(tc[:], base, cb2, op=MUL)
                    nc.vector.tensor_tensor(ts[:], base, sb2, op=MUL)
                    nc.vector.tensor_add(x1, tc[:, :, 0, :], ts[:, :, 1, :])
                    nc.vector.tensor_sub(x2, tc[:, :, 1, :], ts[:, :, 0, :])

                if not TRANS_AFTER:
                    lag = 1 if mt <= 3 else LAG
                    while pending and mt - pending[0][0] >= lag:
                        yield from transpose_tile_pieces(*pending.pop(0))

                # q, k, v rotate through the pp1 PSUM slots; stats/norm/rope
                # drain while later chunks and woven attention pieces run
                for sec in range(2):
                    s_ps = pp1.tile([128, CG], f32, tag="ps1")
                    if QKGEMM_FP8:
                        for g in range(2):
                            nc.tensor.matmul(
                                s_ps[:], qk_x[:, 2 * g:2 * g + 2, :],
                                wqk_sb[:, 2 * g:2 * g + 2,
                                       sec * CG:(sec + 1) * CG],
                                start=(g == 0), stop=False, perf_mode=DR)
                        yield
                        for g in range(2, 4):
                            nc.tensor.matmul(
                                s_ps[:], qk_x[:, 2 * g:2 * g + 2, :],
                                wqk_sb[:, 2 * g:2 * g + 2,
                                       sec * CG:(sec + 1) * CG],
                                start=False, stop=(g == 3), perf_mode=DR)
                    else:
                        for kc in range(4):
                            nc.tensor.matmul(
                                s_ps[:], qk_x[:, kc, :],
                                wqk_sb[:, kc, sec * CG:(sec + 1) * CG],
                                start=(kc == 0), stop=False)
                        yield
                        for kc in range(4, NKC):
                            nc.tensor.matmul(
                                s_ps[:], qk_x[:, kc, :],
                                wqk_sb[:, kc, sec * CG:(sec + 1) * CG],
                                start=False, stop=(kc == NKC - 1))
                    stats_norm_rope(sec, s_ps)
                    yield

                v_ps = pp1.tile([128, CG], f32, tag="ps1")
                for kc in range(4):
                    nc.tensor.matmul(
                        v_ps[:], xt_sb[:, kc, :], wv_sb[:, kc, :],
                        start=(kc == 0), stop=False)
                yield
                for kc in range(4, NKC):
                    nc.tensor.matmul(
                        v_ps[:], xt_sb[:, kc, :], wv_sb[:, kc, :],
                        start=False, stop=(kc == NKC - 1))
                # v -> vaug (strided per-head copy, leaves ones col intact)
                veng = nc.gpsimd if VCOPY_ON_POOL else nc.scalar
                veng.tensor_copy(
                    vaug[:, mt, :, 0:HD],
                    v_ps[:].rearrange("p (h d) -> p h d", d=HD))
                pending.append((mt, qkn))
                yield
                # transposes lag 1-2 tiles, emitted AFTER this tile's
                # sections: the PE must never head-of-line block on the
                # rope chain with ready qkv matmuls queued behind it
                if TRANS_AFTER:
                    lag = 1 if mt <= 3 else LAG
                    while pending and mt - pending[0][0] >= lag:
                        yield from transpose_tile_pieces(*pending.pop(0))
            while pending:
                yield from transpose_tile_pieces(*pending.pop(0))

        def av_win_pieces(h, wi, c0, sz, pt):
            """attn@v + epilogue for a windowed unit, as yieldable pieces."""
            po, tr = (h % 2) * 64, h // 2
            nq = sz // 128
            qt0 = c0 // 128
            yT = yend.tile([128, nq, HD + 1], f32, tag="ye")
            for i in range(nq):
                qt = qt0 + i
                for kt in range(qt + 1):
                    nc.tensor.matmul(
                        yT[:, i, :],
                        pt[:, kt, i * 128:(i + 1) * 128],
                        vaug[:, kt, h, :],
                        start=(kt == 0), stop=(kt == qt))
                if i % 2 == 1 and i != nq - 1:
                    yield
            # divide by denominator (col 64, per-partition, broadcast over d)
            den_b = bass.AP(tensor=yT.tensor, offset=yT.offset + HD,
                            ap=[yT.ap[0], [HD + 1, nq], [0, HD]])
            ysb = epil.tile([128, nq, HD], bf16, tag=f"ysb{nq}")
            nc.gpsimd.tensor_tensor(ysb[:], yT[:, :, 0:HD], den_b,
                                    op=mybir.AluOpType.divide)
            yield
            # back to (ch x T) for the projection
            ytp = yend.tile([64, nq, 128], bf16, tag="ye")
            for i in range(nq):
                nc.tensor.transpose(ytp[:, i, :], ysb[:, i, :], ident[:])
            nc.vector.tensor_copy(
                ybuf[po:po + 64, tr, c0:c0 + sz]
                .rearrange("p (i t) -> p i t", t=128),
                ytp[:])
            state["win_done"][wi] += 1
            yield

        def av_qt_pieces(h, qt, pt3, last):
            """attn@v for a qt-granular unit (tail); epilogue inline after
            the last qt so each head's tail completes as early as possible."""
            yT3 = yend.tile([128, HD + 1], f32, tag="ye")
            for kt in range(qt + 1):
                nc.tensor.matmul(
                    yT3[:], pt3[:, kt, :], vaug[:, kt, h, :],
                    start=(kt == 0), stop=(kt == qt))
            # park the undivided result (and its denominator) in SBUF.
            # Tail pieces run on DVE: the Pool queue backlog would otherwise
            # hold the final barrier
            eng = nc.vector if qt == NMT - 1 else nc.gpsimd
            eng.tensor_copy(y3[:, h, qt - (NMT - NTQ), :], yT3[:])
            yield
            if not last:
                return
            po, tr = (h % 2) * 64, h // 2
            y3h = y3[:, h]
            den_b = bass.AP(tensor=y3h.tensor, offset=y3h.offset + HD,
                            ap=[y3h.ap[0], [HD + 1, NTQ], [0, HD]])
            ysb = epil.tile([128, NTQ, HD], bf16, tag="ysbt")
            nc.vector.tensor_tensor(ysb[:], y3[:, h, :, 0:HD], den_b,
                                    op=mybir.AluOpType.divide)
            yield
            ytp = yend.tile([64, NTQ, 128], bf16, tag="ye")
            for i in range(NTQ):
                nc.tensor.transpose(ytp[:, i, :], ysb[:, i, :], ident[:])
            nc.vector.tensor_copy(
                ybuf[po:po + 64, tr, T - NTQ * 128:T]
                .rearrange("p (i t) -> p i t", t=128),
                ytp[:])
            state["win_done"][NWIN - 1] += 1
            yield

        def attn_gen():
            # software pipeline: while unit k's scores+exp stream, unit k-1's
            # attn@v chains run between them, so the PE never parks waiting
            # for the Act engine's exp of the current unit.
            pend = None  # av piece generator of the previous unit

            def drain_one():
                nonlocal pend
                if pend is not None:
                    try:
                        next(pend)
                    except StopIteration:
                        pend = None

            for wi, (c0, sz) in enumerate(WINS):
                n_kt = (c0 + sz) // 128
                qt0 = c0 // 128
                # one tile of lead (copies drain before the gated scores),
                # except early windows where attention must start ASAP
                need = min(n_kt, NMT - 1) if n_kt > LEAD_NKT else n_kt - 1
                for h in range(HPG):
                    while state["mt_done"] < need:
                        if pend is not None:
                            drain_one()
                        yield "wait"
                    pt = ptpool.tile([128, 14, QW], bf16, tag="pt")
                    for g in range((n_kt + 1) // 2):
                        st = stps.tile([128, 2, QW], f32, tag="st")
                        kts = [kt for kt in (2 * g, 2 * g + 1) if kt < n_kt]
                        for j, kt in enumerate(kts):
                            d = kt - qt0
                            col0 = d * 128 if d >= 0 else 0
                            score_mm(st[:, j, col0:col0 + (sz - col0)],
                                     h, kt, c0 + col0, c0 + sz)
                        # batched exp; stale PSUM cols left of the causal
                        # edge are exp'd but never read
                        ec = min(max(0, (2 * g - qt0) * 128), sz - 128)
                        nc.scalar.activation(
                            pt[:, 2 * g:2 * g + len(kts), ec:sz],
                            st[:, 0:len(kts), ec:sz],
                            AF.Exp, scale=ESC)
                        for kt in kts:
                            d = kt - qt0
                            if d >= 0:
                                col0 = d * 128
                                nc.gpsimd.tensor_tensor(
                                    pt[:, kt, col0:col0 + 128],
                                    pt[:, kt, col0:col0 + 128], trim_sb[:],
                                    op=MUL)
                        drain_one()
                        yield
                    while pend is not None:
                        drain_one()
                        yield
                    pend = av_win_pieces(h, wi, c0, sz, pt)

            # ---- tail: query-tile granular ----
            for qt in range(NMT - NTQ, NMT):
                for h in range(HPG):
                    while state["mt_done"] < qt:
                        if pend is not None:
                            drain_one()
                        yield "wait"
                    pt3 = ptpool.tile([128, NKT, 128], bf16, tag="pt3")
                    for b in range(2):
                        kts = list(range(8 * b, min(8 * b + 8, qt + 1)))
                        st = stps.tile([128, 8, 128], f32, tag="st")
                        for i, kt in enumerate(kts):
                            score_mm(st[:, i, :], h, kt,
                                     qt * 128, (qt + 1) * 128)
                        nc.scalar.activation(
                            pt3[:, 8 * b:8 * b + len(kts), :],
                            st[:, 0:len(kts), :],
                            AF.Exp, scale=ESC)
                        if qt in kts:
                            nc.gpsimd.tensor_tensor(
                                pt3[:, qt, :], pt3[:, qt, :], trim_sb[:],
                                op=MUL)
                        drain_one()
                        yield
                    while pend is not None:
                        drain_one()
                        yield
                    pend = av_qt_pieces(h, qt, pt3, last=(qt == NMT - 1))
            while pend is not None:
                drain_one()
                yield

        def proj_gen():
            pw = WINS + [(T - NTQ * 128, NTQ * 128)]
            for (wi, (c0, sz)), mo in [((wi, w), mo) for wi, w in enumerate(pw)
                                       for mo in range(C // 128)]:
                # also wait out the qkv phase: proj's matmuls, staging
                # copies and DMAs would steal DVE/Pool/PE from the
                # DVE-bound qkv pipeline
                while state["win_done"][wi] < HPG or state["mt_done"] < 13:
                    yield "wait"
                po_ps = pp1.tile([128, QW], f32, tag="ps1")
                for kc in range(CG // 128):
                    nc.tensor.matmul(
                        po_ps[:, 0:sz],
                        wp_sb[:, kc, mo * 128:(mo + 1) * 128],
                        ybuf[:, kc, c0:c0 + sz],
                        start=(kc == 0), stop=(kc == CG // 128 - 1))
                ot = ostg.tile([128, QW], f32)
                # alternate engines so neither queue serializes the drain;
                # the last window avoids Pool (barrier waits on its backlog)
                ceng = nc.gpsimd if (mo % 2 == 0 and wi < NWIN - 1) \
                    else nc.vector
                ceng.tensor_copy(ot[:, 0:sz], po_ps[:, 0:sz])
                nc.sync.dma_start(
                    out[mo * 128:(mo + 1) * 128, c0:c0 + sz],
                    ot[:, 0:sz])
                yield

        # weave: one p1/proj piece, then up to two attention pieces
        g_p1, g_at, g_pj = p1_gen(), attn_gen(), proj_gen()

        def step(g):
            if g is None:
                return None, False
            try:
                r = next(g)
                return g, r != "wait"
            except StopIteration:
                return None, False

        while g_p1 is not None or g_at is not None or g_pj is not None:
            progressed = False
            if g_p1 is not None:
                g_p1, ok = step(g_p1)
                progressed |= ok
            else:
                g_pj, ok = step(g_pj)
                progressed |= ok
            for _ in range(ATTN_PER_ROUND):
                g_at, ok = step(g_at)
                progressed |= ok
                if g_at is None:
                    break
            if not progressed and g_p1 is None and g_at is None:
                # drain remaining proj
                while g_pj is not None:
                    g_pj, _ = step(g_pj)

    _split_excess_waits(nc, mybir)
    return nc


_NC_CACHE = {}


def _get_nc():
    if "nc" not in _NC_CACHE:
        _NC_CACHE["nc"] = _build_nc()
    return _NC_CACHE["nc"]


def _host_inputs(x, w_attn, w_proj):
    import ml_dtypes
    bf = ml_dtypes.bfloat16
    f8 = ml_dtypes.float8_e4m3
    inv_freq = 1.0 / (10000.0 ** (np.arange(0, HD, 2, dtype=np.float32) / HD))
    t = np.arange(T, dtype=np.float32)
    freqs = np.outer(t, inv_freq)
    cos = np.cos(freqs).astype(bf)
    sin = np.sin(freqs).astype(bf)
    cosn = np.ascontiguousarray(cos.reshape(NMT, 128, HD // 2).transpose(1, 0, 2))
    sinn = np.ascontiguousarray(sin.reshape(NMT, 128, HD // 2).transpose(1, 0, 2))
    trim = np.triu(np.ones((128, 128), dtype=np.float32)).astype(bf)

    in_maps = []
    for b in range(B):
        xT = np.ascontiguousarray(x[b].T)  # (C, T)
        # [mt, ch-in-chunk, kc*128+tok]: one contiguous DMA per token tile,
        # partitions carry the contraction channels
        xt_f = np.ascontiguousarray(
            xT.reshape(NKC, 128, NMT, 128).transpose(2, 1, 0, 3)
        ).reshape(NMT, 128, NKC * 128)
        xt = xt_f.astype(bf)
        xt8 = xt_f.astype(f8)
        for hg in range(HG):
            qr = slice(hg * CG, (hg + 1) * CG)
            kr = slice(C + hg * CG, C + (hg + 1) * CG)
            vr = slice(2 * C + hg * CG, 2 * C + (hg + 1) * CG)
            wqk_f = np.ascontiguousarray(
                np.concatenate([w_attn[qr], w_attn[kr]], axis=0).T)
            if QKGEMM_FP8:
                wqk_h = (wqk_f * WSCALE).astype(f8)
            else:
                wqk_h = wqk_f.astype(bf)
            wv = np.ascontiguousarray(w_attn[vr].T).astype(bf)
            wp = np.ascontiguousarray(w_proj[:, hg * CG:(hg + 1) * CG].T).astype(bf)
            m = {
                "xt": xt, "wqk": wqk_h, "wv": wv, "wp": wp,
                "cosn": cosn, "sinn": sinn, "trim": trim,
            }
            if QKGEMM_FP8:
                m["xt8"] = xt8
            in_maps.append(m)
    return in_maps


def kernel(x, w_attn, w_proj, _profile=False):
    from concourse.bass_utils import run_bass_kernel_spmd
    nc = _get_nc()
    in_maps = _host_inputs(
        np.asarray(x, dtype=np.float32),
        np.asarray(w_attn, dtype=np.float32),
        np.asarray(w_proj, dtype=np.float32))
    res = run_bass_kernel_spmd(nc, in_maps, core_ids=list(range(N_CORES)),
                               trace=_profile)
    out = np.empty((B, T, C), dtype=np.float32)
    for b in range(B):
        acc = res.results[2 * b]["o"] + res.results[2 * b + 1]["o"]
        out[b] = acc.T
    if _profile:
        return out, res
    return out
